# revision 18
# baseline (speedup 1.0000x reference)
"""MultiHeadAttention Trainium2 kernel.

B=4, T=2048, D=512, H=8 heads (head dim 64). 8 NeuronCores.

Sharding: core i handles batch b = i//2, query rows half = i%2 (1024 rows).
Each core computes its full attention + output projection slice; outputs are
disjoint so the host just concatenates (no collectives).

Host prep (not counted in HW exec time):
  - q/k/v transposed to [D, t] layout (matmul-native; avoids on-device
    transposes, which fp32 DMA-transpose can't do anyway).
  - k/v compacted to only the unmasked key positions per batch (masked
    softmax weights are exactly 0 in the reference since exp(-65504-max)
    underflows, so dropping those columns is mathematically exact). Padded
    to a multiple of 128; padded columns are excluded from the softmax
    denominator via a 0/1 "valid" column carried next to v.

Device per core (all matmuls float32r = full PE rate, fp32 storage):
  qh_T[c, tq] = Wq @ q.T  (scale folded in), kh_T[c, tk], vh[tk, c] + valid col
  per head: scores_T[tk, tq] = kh_T.T-slice matmuls (K=64, head pairs row-tiled)
            a_T = exp(scores_T)        (ScalarE, PSUM -> SBUF)
            o_aug[c+1, tq] += [vh | valid].T @ a_T   (ones col => softmax sums)
            o = o_aug[:64] * (1 / o_aug[64])  (DVE recip + gpsimd partition bcast)
  out[tq, d] = sum_h o_h.T @ Wo_h.T + bo  (bias via K=1 ones-row matmul)
"""

import numpy as np
from functools import lru_cache

import concourse.bacc as bacc
import concourse.mybir as mybir
import concourse.tile as tile
from concourse.bass_utils import run_bass_kernel_spmd

P = 128
D = 512
NH = 8
C = 64
TQ = 1024  # query rows per core
B, T = 4, 2048
N_CORES = 8
F32 = mybir.dt.float32
F32R = mybir.dt.float32r
F16 = mybir.dt.float16
EXP = mybir.ActivationFunctionType.Exp
SCALE = float(D) ** -0.5


@lru_cache(maxsize=8)
def _build(KP: int, dbg: bool = False):
    """Build + compile the SPMD program for padded key count KP."""
    NK = KP // P
    nc = bacc.Bacc(None, target_bir_lowering=False, debug=False)
    dbg_d = {}
    if dbg:
        for nm2, shp, dt_ in (("d_qhT0", [P, TQ], F16), ("d_khT0", [P, KP], F16),
                              ("d_vh0", [P, NH * (C + 1)], F16),
                              ("d_aT", [P, 2 * D], F16),
                              ("d_onT0", [C, TQ], F16),
                              ("d_osb", [C + 1, D], F32),
                              ("d_rrep", [C, D], F32)):
            dbg_d[nm2] = nc.dram_tensor(nm2, shp, dt_, kind="ExternalOutput")

    qt_d = nc.dram_tensor("qt", [D, TQ], F16, kind="ExternalInput")
    kt_d = nc.dram_tensor("kt", [D, KP], F16, kind="ExternalInput")
    vt_d = nc.dram_tensor("vt", [D, KP], F16, kind="ExternalInput")
    wq_d = nc.dram_tensor("wqt", [D, D], F16, kind="ExternalInput")
    wk_d = nc.dram_tensor("wkt", [D, D], F16, kind="ExternalInput")
    wv_d = nc.dram_tensor("wvt", [D, D], F16, kind="ExternalInput")
    wo_d = nc.dram_tensor("wot", [D, D], F16, kind="ExternalInput")
    bias_d = nc.dram_tensor("biases", [1, 4 * D + TQ], F16, kind="ExternalInput")
    val_d = nc.dram_tensor("valid", [KP, NH, 1], F16, kind="ExternalInput")
    valc_d = nc.dram_tensor("validc", [KP, 1], F32, kind="ExternalInput")
    bcol_d = nc.dram_tensor("biascol", [P, 8], F32, kind="ExternalInput")
    out_d = nc.dram_tensor("out", [TQ, D], F32, kind="ExternalOutput")

    with tile.TileContext(nc) as tc:
        with (
            tc.tile_pool(name="wp", bufs=12) as wp,
            tc.tile_pool(name="cst", bufs=1) as cst,
            tc.tile_pool(name="xt", bufs=8) as xtp,
            tc.tile_pool(name="pj", bufs=1) as pjp,
            tc.tile_pool(name="vp", bufs=1) as vpp,
            tc.tile_pool(name="at", bufs=3) as atp,
            tc.tile_pool(name="nm", bufs=4) as nmp,
            tc.tile_pool(name="ot", bufs=2) as otp,
            tc.tile_pool(name="ps", bufs=2, space="PSUM") as psp,
        ):
            # ---- constants ----
            bias_sb = cst.tile([1, 4 * D + TQ], F16, tag="bias", name="bias_sb")
            nc.sync.dma_start(out=bias_sb, in_=bias_d[:])
            ones = bias_sb[0:1, 4 * D:4 * D + TQ]
            onescol = cst.tile([1, C], F16, tag="onescol", name="onescol")
            nc.sync.dma_start(out=onescol, in_=bias_d[0:1, 4 * D:4 * D + C])
            bcol = cst.tile([P, 8], F32, tag="bcol", name="bcol")
            nc.sync.dma_start(out=bcol, in_=bcol_d[:])

            wq, wk, wv = [], [], []
            for nm_, lst, dr in (("wq", wq, wq_d), ("wk", wk, wk_d), ("wv", wv, wv_d)):
                for kk in range(4):
                    t = wp.tile([P, D], F16, tag="w", name=f"{nm_}{kk}")
                    nc.sync.dma_start(out=t, in_=dr[kk * P:(kk + 1) * P, :])
                    lst.append(t)

            # ---- x.T loads ----
            qt = []
            for kk in range(4):
                t = xtp.tile([P, TQ], F16, tag="xt", name=f"qt{kk}")
                nc.sync.dma_start(out=t, in_=qt_d[kk * P:(kk + 1) * P, :])
                qt.append(t)
            kt = []
            for kk in range(4):
                t = xtp.tile([P, KP], F16, tag="xt", name=f"kt{kk}")
                nc.sync.dma_start(out=t, in_=kt_d[kk * P:(kk + 1) * P, :])
                kt.append(t)

            # ---- phase 1a: qh_T [c_all, TQ], scale folded in ----
            qhT = [pjp.tile([P, TQ], F16, tag=f"qhT{m}", name=f"qhT{m}") for m in range(4)]
            for m in range(4):
                for t2 in range(2):
                    ps = psp.tile([P, D], F32, tag="ob", name="pj_ps")
                    for kk in range(4):
                        nc.tensor.matmul(
                            ps, wq[kk][:, m * P:(m + 1) * P],
                            qt[kk][:, t2 * D:(t2 + 1) * D],
                            start=(kk == 0), stop=(kk == 3))
                    nc.vector.tensor_scalar(
                        qhT[m][:, t2 * D:(t2 + 1) * D], ps,
                        bcol[:, m:m + 1], SCALE,
                        op0=mybir.AluOpType.add, op1=mybir.AluOpType.mult)

            # ---- phase 1b: kh_T [c_all, KP] ----
            khT = [pjp.tile([P, KP], F16, tag=f"khT{m}", name=f"khT{m}") for m in range(4)]
            for m in range(4):
                for t0 in range(0, KP, D):
                    tw = min(D, KP - t0)
                    ps = psp.tile([P, D], F32, tag="ob", name="pj_ps")
                    for kk in range(4):
                        nc.tensor.matmul(
                            ps[:, :tw], wk[kk][:, m * P:(m + 1) * P],
                            kt[kk][:, t0:t0 + tw],
                            start=(kk == 0), stop=(kk == 3))
                    nc.vector.tensor_scalar_add(
                        khT[m][:, t0:t0 + tw], ps[:, :tw], bcol[:, 4 + m:5 + m])

            # ---- vt loads (reuse qt slots) + phase 1c: vh [tk, heads, 65] ----
            vt = []
            for kk in range(4):
                t = xtp.tile([P, KP], F16, tag="xt", name=f"vt{kk}")
                nc.sync.dma_start(out=t, in_=vt_d[kk * P:(kk + 1) * P, :])
                vt.append(t)
            vh = []
            for n in range(NK):
                ps = psp.tile([P, D], F32, tag="ob", name="vh_ps")
                for kk in range(4):
                    nc.tensor.matmul(
                        ps, vt[kk][:, n * P:(n + 1) * P], wv[kk],
                        start=(kk == 0), stop=False)
                nc.tensor.matmul(
                    ps, ones[:, 0:P], bias_sb[0:1, 2 * D:3 * D],
                    start=False, stop=True)
                vh_n = vpp.tile([P, NH, C + 1], F16, tag=f"vh{n}", name=f"vh{n}")
                valc = vpp.tile([P, 1], F32, tag=f"valc{n}", name=f"valc{n}")
                nc.sync.dma_start(out=valc, in_=valc_d[n * P:(n + 1) * P, :])
                # valid-scaled copy: zeroes padded v rows (bias would otherwise
                # leak into the numerator through padding)
                nc.vector.tensor_scalar_mul(
                    vh_n[:, :, 0:C], ps.rearrange("p (h c) -> p h c", h=NH), valc)
                nc.sync.dma_start(
                    out=vh_n[:, :, C:C + 1], in_=val_d[n * P:(n + 1) * P, :, :])
                vh.append(vh_n)

            if dbg:
                nc.sync.dma_start(out=dbg_d["d_qhT0"][:], in_=qhT[0])
                nc.sync.dma_start(out=dbg_d["d_khT0"][:], in_=khT[0])
                nc.sync.dma_start(
                    out=dbg_d["d_vh0"][:],
                    in_=vh[0].rearrange("p h c -> p (h c)"))

            # ---- phase 2: attention (with progressive output projection) ----
            wo = []
            for j in range(NH // 2):
                t = wp.tile([P, D], F16, tag="w", name=f"wo{j}")
                nc.sync.dma_start(out=t, in_=wo_d[j * P:(j + 1) * P, :])
                wo.append(t)
            out_acc = [otp.tile([P, D], F32, tag=f"out_acc{tqc}",
                                name=f"out_acc{tqc}", bufs=1) for tqc in range(8)]
            onTp = [nmp.tile([P, TQ], F16, tag=f"onTp{j}", name=f"onTp{j}", bufs=1)
                    for j in range(NH // 2)]
            for t2 in range(2):
                tsl = slice(t2 * D, (t2 + 1) * D)
                for hp in range(4):
                    h0, h1 = 2 * hp, 2 * hp + 1
                    o_ps = psp.tile([C + 1, 2 * D], F32, tag="ob", name="o_ps")
                    for n in range(NK):
                        s = psp.tile([P, 2 * D], F32, tag="big", name="s_ps")
                        nc.tensor.matmul(
                            s[:, 0:D],
                            khT[hp][0:C, n * P:(n + 1) * P],
                            qhT[hp][0:C, tsl], start=True, stop=True)
                        nc.tensor.matmul(
                            s[:, D:2 * D],
                            khT[hp][C:P, n * P:(n + 1) * P],
                            qhT[hp][C:P, tsl], start=True, stop=True)
                        a = atp.tile([P, 2 * D], F16, tag="aT", name="aT")
                        nc.scalar.activation(a, s, EXP)
                        if dbg and t2 == 0 and hp == 0 and n == 0:
                            nc.sync.dma_start(out=dbg_d["d_aT"][:], in_=a)
                        nc.tensor.matmul(
                            o_ps[:, 0:D], vh[n][:, h0, :], a[:, 0:D],
                            start=(n == 0), stop=(n == NK - 1))
                        nc.tensor.matmul(
                            o_ps[:, D:2 * D], vh[n][:, h1, :], a[:, D:2 * D],
                            start=(n == 0), stop=(n == NK - 1))
                    for j, hh in ((0, h0), (1, h1)):
                        osl = slice(j * D, (j + 1) * D)
                        osb = nmp.tile([C + 1, D], F32, tag="osb", name="osb")
                        nc.vector.tensor_copy(osb, o_ps[:, osl])
                        # move the sums row to partition 0 (custom DVE ops and
                        # the ones-row matmul operands must be base-0 on HW)
                        rv = nmp.tile([1, D], F32, tag="rv", name="rv", bufs=2)
                        nc.vector.tensor_copy(rv, osb[C:C + 1, :])
                        rcp = nmp.tile([1, D], F32, tag="rcp", name="rcp", bufs=2)
                        nc.vector.reciprocal_approx_fast(out=rcp, in_=rv)
                        rrow = nmp.tile([1, D], F16, tag="rrow", name="rrow", bufs=2)
                        nc.vector.tensor_copy(rrow, rcp)
                        # replicate 1/sum across partitions via PE outer product
                        rrep_ps = psp.tile([C, D], F32, tag="ob", name="rrep_ps")
                        nc.tensor.matmul(rrep_ps, onescol, rrow,
                                         start=True, stop=True)
                        nc.vector.tensor_mul(
                            onTp[hh // 2][(hh % 2) * C:(hh % 2) * C + C, tsl],
                            osb[0:C, :], rrep_ps)
                    if t2 == 1:
                        # pair hp's onTp is final: fold its output-projection
                        # contribution into out_acc now (overlaps later pairs)
                        for tqc in range(8):
                            pps = psp.tile([P, D], F32, tag="ob", name="p3_ps")
                            nc.tensor.matmul(
                                pps, onTp[hp][:, tqc * P:(tqc + 1) * P], wo[hp],
                                start=True, stop=(hp != 0))
                            if hp == 0:
                                nc.tensor.matmul(
                                    pps, ones[:, 0:P],
                                    bias_sb[0:1, 3 * D:4 * D],
                                    start=False, stop=True)
                                nc.vector.tensor_copy(out_acc[tqc], pps)
                            else:
                                nc.vector.tensor_add(
                                    out_acc[tqc], out_acc[tqc], pps)
                        if dbg and t2 == 0 and hh == 0:
                            nc.sync.dma_start(out=dbg_d["d_osb"][:], in_=osb)
                            rrep_sb = nmp.tile([C, D], F32, tag="rrep_dbg",
                                               name="rrep_dbg", bufs=1)
                            nc.vector.tensor_copy(rrep_sb, rrep_ps)
                            nc.sync.dma_start(out=dbg_d["d_rrep"][:], in_=rrep_sb)

            if dbg:
                nc.sync.dma_start(out=dbg_d["d_onT0"][:], in_=onTp[0][0:C, :])

            # ---- phase 3: store accumulated output ----
            for tqc in range(8):
                nc.sync.dma_start(
                    out=out_d[tqc * P:(tqc + 1) * P, :], in_=out_acc[tqc])

    nc.compile()
    return nc


def _prep(q, k, v, mask, Wq, bq, Wk, bk, Wv, bv, Wo, bo):
    q = np.asarray(q, np.float32)
    k = np.asarray(k, np.float32)
    v = np.asarray(v, np.float32)
    mask = np.asarray(mask)
    wqt = np.ascontiguousarray(np.asarray(Wq, np.float32).T.astype(np.float16))
    wkt = np.ascontiguousarray(np.asarray(Wk, np.float32).T.astype(np.float16))
    wvt = np.ascontiguousarray(np.asarray(Wv, np.float32).T.astype(np.float16))
    wot = np.ascontiguousarray(np.asarray(Wo, np.float32).T.astype(np.float16))
    biascol = np.concatenate([
        np.asarray(bq, np.float32).reshape(4, P).T,
        np.asarray(bk, np.float32).reshape(4, P).T], axis=1)
    biascol = np.ascontiguousarray(biascol, dtype=np.float32)
    biases = np.concatenate(
        [np.asarray(x, np.float32) for x in (bq, bk, bv, bo)]
        + [np.ones(TQ, np.float32)]).reshape(1, 4 * D + TQ).astype(np.float16)

    sels = [np.flatnonzero(mask[b]) for b in range(B)]
    kmax = max(1, max(len(s) for s in sels))
    KP = ((kmax + P - 1) // P) * P

    in_maps = []
    for core in range(N_CORES):
        b, half = divmod(core, 2)
        sel = sels[b]
        ns = len(sel)
        kt = np.zeros((D, KP), np.float16)
        kt[:, :ns] = k[b, sel, :].T
        vt = np.zeros((D, KP), np.float16)
        vt[:, :ns] = v[b, sel, :].T
        valid = np.zeros((KP, NH, 1), np.float16)
        valid[:ns] = 1.0
        validc = np.zeros((KP, 1), np.float32)
        validc[:ns] = 1.0
        qt = np.ascontiguousarray(
            q[b, half * TQ:(half + 1) * TQ, :].T.astype(np.float16))
        in_maps.append(dict(
            qt=qt, kt=kt, vt=vt, wqt=wqt, wkt=wkt, wvt=wvt, wot=wot,
            biases=biases, valid=valid, validc=validc, biascol=biascol))
    return KP, in_maps


def kernel(q, k, v, mask, Wq, bq, Wk, bk, Wv, bv, Wo, bo, _bench=[None]):
    KP, in_maps = _prep(q, k, v, mask, Wq, bq, Wk, bk, Wv, bv, Wo, bo)
    nc = _build(KP)
    res = run_bass_kernel_spmd(nc, in_maps, list(range(N_CORES)))
    _bench[0] = res
    out = np.empty((B, T, D), np.float32)
    for core in range(N_CORES):
        b, half = divmod(core, 2)
        out[b, half * TQ:(half + 1) * TQ, :] = res.results[core]["out"]
    return out


# revision 20
# speedup vs baseline: 1.0892x; 1.0892x over previous
"""MultiHeadAttention Trainium2 kernel.

B=4, T=2048, D=512, H=8 heads (head dim 64). 8 NeuronCores.

Sharding: core i handles batch b = i//2, query rows half = i%2 (1024 rows).
Each core computes its full attention + output projection slice; outputs are
disjoint so the host just concatenates (no collectives).

Host prep (not counted in HW exec time):
  - q/k/v transposed to [D, t] layout (matmul-native; avoids on-device
    transposes, which fp32 DMA-transpose can't do anyway).
  - k/v compacted to only the unmasked key positions per batch (masked
    softmax weights are exactly 0 in the reference since exp(-65504-max)
    underflows, so dropping those columns is mathematically exact). Padded
    to a multiple of 128; padded columns are excluded from the softmax
    denominator via a 0/1 "valid" column carried next to v.

Device per core (all matmuls float32r = full PE rate, fp32 storage):
  qh_T[c, tq] = Wq @ q.T  (scale folded in), kh_T[c, tk], vh[tk, c] + valid col
  per head: scores_T[tk, tq] = kh_T.T-slice matmuls (K=64, head pairs row-tiled)
            a_T = exp(scores_T)        (ScalarE, PSUM -> SBUF)
            o_aug[c+1, tq] += [vh | valid].T @ a_T   (ones col => softmax sums)
            o = o_aug[:64] * (1 / o_aug[64])  (DVE recip + gpsimd partition bcast)
  out[tq, d] = sum_h o_h.T @ Wo_h.T + bo  (bias via K=1 ones-row matmul)
"""

import numpy as np
from functools import lru_cache

import concourse.bacc as bacc
import concourse.mybir as mybir
import concourse.tile as tile
from concourse.bass_utils import run_bass_kernel_spmd

P = 128
D = 512
NH = 8
C = 64
TQ = 1024  # query rows per core
B, T = 4, 2048
N_CORES = 8
F32 = mybir.dt.float32
F32R = mybir.dt.float32r
F16 = mybir.dt.float16
EXP = mybir.ActivationFunctionType.Exp
SCALE = float(D) ** -0.5


@lru_cache(maxsize=8)
def _build(KP: int, dbg: bool = False, use_bias: bool = False):
    """Build + compile the SPMD program for padded key count KP."""
    NK = KP // P
    nc = bacc.Bacc(None, target_bir_lowering=False, debug=False)
    dbg_d = {}
    if dbg:
        for nm2, shp, dt_ in (("d_qhT0", [P, TQ], F16), ("d_khT0", [P, KP], F16),
                              ("d_vh0", [P, NH * (C + 1)], F16),
                              ("d_aT", [P, 2 * D], F16),
                              ("d_onT0", [C, TQ], F16),
                              ("d_osb", [C + 1, D], F32),
                              ("d_rrep", [C, D], F32)):
            dbg_d[nm2] = nc.dram_tensor(nm2, shp, dt_, kind="ExternalOutput")

    qt_d = nc.dram_tensor("qt", [D, TQ], F16, kind="ExternalInput")
    kt_d = nc.dram_tensor("kt", [D, KP], F16, kind="ExternalInput")
    vt_d = nc.dram_tensor("vt", [D, KP], F16, kind="ExternalInput")
    wq_d = nc.dram_tensor("wqt", [D, D], F16, kind="ExternalInput")
    wk_d = nc.dram_tensor("wkt", [D, D], F16, kind="ExternalInput")
    wv_d = nc.dram_tensor("wvt", [D, D], F16, kind="ExternalInput")
    wo_d = nc.dram_tensor("wot", [D, D], F16, kind="ExternalInput")
    bias_d = nc.dram_tensor("biases", [1, 4 * D + TQ], F16, kind="ExternalInput")
    val_d = nc.dram_tensor("valid", [KP, NH, 1], F16, kind="ExternalInput")
    valc_d = nc.dram_tensor("validc", [KP, 1], F32, kind="ExternalInput")
    bcol_d = nc.dram_tensor("biascol", [P, 8], F32, kind="ExternalInput")
    out_d = nc.dram_tensor("out", [TQ, D], F32, kind="ExternalOutput")

    with tile.TileContext(nc) as tc:
        with (
            tc.tile_pool(name="wp", bufs=12) as wp,
            tc.tile_pool(name="cst", bufs=1) as cst,
            tc.tile_pool(name="xt", bufs=8) as xtp,
            tc.tile_pool(name="pj", bufs=1) as pjp,
            tc.tile_pool(name="vp", bufs=1) as vpp,
            tc.tile_pool(name="at", bufs=3) as atp,
            tc.tile_pool(name="nm", bufs=4) as nmp,
            tc.tile_pool(name="ot", bufs=2) as otp,
            tc.tile_pool(name="ps", bufs=2, space="PSUM") as psp,
        ):
            # ---- constants ----
            bias_sb = cst.tile([1, 4 * D + TQ], F16, tag="bias", name="bias_sb")
            nc.sync.dma_start(out=bias_sb, in_=bias_d[:])
            ones = bias_sb[0:1, 4 * D:4 * D + TQ]
            onescol = cst.tile([1, C], F16, tag="onescol", name="onescol")
            nc.sync.dma_start(out=onescol, in_=bias_d[0:1, 4 * D:4 * D + C])
            bcol = cst.tile([P, 8], F32, tag="bcol", name="bcol")
            nc.sync.dma_start(out=bcol, in_=bcol_d[:])

            wq, wk, wv = [], [], []
            for nm_, lst, dr in (("wq", wq, wq_d), ("wk", wk, wk_d), ("wv", wv, wv_d)):
                for kk in range(4):
                    t = wp.tile([P, D], F16, tag="w", name=f"{nm_}{kk}")
                    nc.sync.dma_start(out=t, in_=dr[kk * P:(kk + 1) * P, :])
                    lst.append(t)

            # ---- x.T loads ----
            qt = []
            for kk in range(4):
                t = xtp.tile([P, TQ], F16, tag="xt", name=f"qt{kk}")
                nc.sync.dma_start(out=t, in_=qt_d[kk * P:(kk + 1) * P, :])
                qt.append(t)
            kt = []
            for kk in range(4):
                t = xtp.tile([P, KP], F16, tag="xt", name=f"kt{kk}")
                nc.sync.dma_start(out=t, in_=kt_d[kk * P:(kk + 1) * P, :])
                kt.append(t)

            # ---- phase 1a: qh_T [c_all, TQ], scale folded in ----
            qhT = [pjp.tile([P, TQ], F16, tag=f"qhT{m}", name=f"qhT{m}") for m in range(4)]
            for m in range(4):
                for t2 in range(2):
                    ps = psp.tile([P, D], F32, tag="ob", name="pj_ps")
                    for kk in range(4):
                        nc.tensor.matmul(
                            ps, wq[kk][:, m * P:(m + 1) * P],
                            qt[kk][:, t2 * D:(t2 + 1) * D],
                            start=(kk == 0), stop=(kk == 3))
                    if use_bias:
                        nc.vector.tensor_scalar(
                            qhT[m][:, t2 * D:(t2 + 1) * D], ps,
                            bcol[:, m:m + 1], SCALE,
                            op0=mybir.AluOpType.add, op1=mybir.AluOpType.mult)
                    else:
                        nc.vector.tensor_scalar_mul(
                            qhT[m][:, t2 * D:(t2 + 1) * D], ps, SCALE)

            # ---- phase 1b: kh_T [c_all, KP] ----
            khT = [pjp.tile([P, KP], F16, tag=f"khT{m}", name=f"khT{m}") for m in range(4)]
            for m in range(4):
                for t0 in range(0, KP, D):
                    tw = min(D, KP - t0)
                    ps = psp.tile([P, D], F32, tag="ob", name="pj_ps")
                    for kk in range(4):
                        nc.tensor.matmul(
                            ps[:, :tw], wk[kk][:, m * P:(m + 1) * P],
                            kt[kk][:, t0:t0 + tw],
                            start=(kk == 0), stop=(kk == 3))
                    if use_bias:
                        nc.vector.tensor_scalar_add(
                            khT[m][:, t0:t0 + tw], ps[:, :tw], bcol[:, 4 + m:5 + m])
                    else:
                        nc.vector.tensor_copy(khT[m][:, t0:t0 + tw], ps[:, :tw])

            # ---- vt loads (reuse qt slots) + phase 1c: vh [tk, heads, 65] ----
            vt = []
            for kk in range(4):
                t = xtp.tile([P, KP], F16, tag="xt", name=f"vt{kk}")
                nc.sync.dma_start(out=t, in_=vt_d[kk * P:(kk + 1) * P, :])
                vt.append(t)
            vh = []
            for n in range(NK):
                ps = psp.tile([P, D], F32, tag="ob", name="vh_ps")
                for kk in range(4):
                    nc.tensor.matmul(
                        ps, vt[kk][:, n * P:(n + 1) * P], wv[kk],
                        start=(kk == 0), stop=(kk == 3 and not use_bias))
                if use_bias:
                    nc.tensor.matmul(
                        ps, ones[:, 0:P], bias_sb[0:1, 2 * D:3 * D],
                        start=False, stop=True)
                vh_n = vpp.tile([P, NH, C + 1], F16, tag=f"vh{n}", name=f"vh{n}")
                valc = vpp.tile([P, 1], F32, tag=f"valc{n}", name=f"valc{n}")
                nc.sync.dma_start(out=valc, in_=valc_d[n * P:(n + 1) * P, :])
                # valid-scaled copy: zeroes padded v rows (bias would otherwise
                # leak into the numerator through padding)
                nc.vector.tensor_scalar_mul(
                    vh_n[:, :, 0:C], ps.rearrange("p (h c) -> p h c", h=NH), valc)
                nc.sync.dma_start(
                    out=vh_n[:, :, C:C + 1], in_=val_d[n * P:(n + 1) * P, :, :])
                vh.append(vh_n)

            if dbg:
                nc.sync.dma_start(out=dbg_d["d_qhT0"][:], in_=qhT[0])
                nc.sync.dma_start(out=dbg_d["d_khT0"][:], in_=khT[0])
                nc.sync.dma_start(
                    out=dbg_d["d_vh0"][:],
                    in_=vh[0].rearrange("p h c -> p (h c)"))

            # ---- phase 2: attention (with progressive output projection) ----
            wo = []
            for j in range(NH // 2):
                t = wp.tile([P, D], F16, tag="w", name=f"wo{j}")
                nc.sync.dma_start(out=t, in_=wo_d[j * P:(j + 1) * P, :])
                wo.append(t)
            onTp = [nmp.tile([P, TQ], F16, tag=f"onTp{j}", name=f"onTp{j}", bufs=1)
                    for j in range(NH // 2)]
            for t2 in range(2):
                tsl = slice(t2 * D, (t2 + 1) * D)
                for hp in range(4):
                    h0, h1 = 2 * hp, 2 * hp + 1
                    o_ps = psp.tile([C + 1, 2 * D], F32, tag="ob", name="o_ps")
                    for n in range(NK):
                        s = psp.tile([P, 2 * D], F32, tag="big", name="s_ps")
                        nc.tensor.matmul(
                            s[:, 0:D],
                            khT[hp][0:C, n * P:(n + 1) * P],
                            qhT[hp][0:C, tsl], start=True, stop=True)
                        nc.tensor.matmul(
                            s[:, D:2 * D],
                            khT[hp][C:P, n * P:(n + 1) * P],
                            qhT[hp][C:P, tsl], start=True, stop=True)
                        a = atp.tile([P, 2 * D], F16, tag="aT", name="aT")
                        nc.scalar.activation(a, s, EXP)
                        if dbg and t2 == 0 and hp == 0 and n == 0:
                            nc.sync.dma_start(out=dbg_d["d_aT"][:], in_=a)
                        nc.tensor.matmul(
                            o_ps[:, 0:D], vh[n][:, h0, :], a[:, 0:D],
                            start=(n == 0), stop=(n == NK - 1))
                        nc.tensor.matmul(
                            o_ps[:, D:2 * D], vh[n][:, h1, :], a[:, D:2 * D],
                            start=(n == 0), stop=(n == NK - 1))
                    for j, hh in ((0, h0), (1, h1)):
                        osl = slice(j * D, (j + 1) * D)
                        osb = nmp.tile([C + 1, D], F32, tag="osb", name="osb")
                        nc.vector.tensor_copy(osb, o_ps[:, osl])
                        # move the sums row to partition 0 (custom DVE ops and
                        # the ones-row matmul operands must be base-0 on HW)
                        rv = nmp.tile([1, D], F32, tag="rv", name="rv", bufs=2)
                        nc.vector.tensor_copy(rv, osb[C:C + 1, :])
                        rcp = nmp.tile([1, D], F32, tag="rcp", name="rcp", bufs=2)
                        nc.vector.reciprocal_approx_fast(out=rcp, in_=rv)
                        rrow = nmp.tile([1, D], F16, tag="rrow", name="rrow", bufs=2)
                        nc.vector.tensor_copy(rrow, rcp)
                        # replicate 1/sum across partitions via PE outer product
                        rrep_ps = psp.tile([C, D], F32, tag="ob", name="rrep_ps")
                        nc.tensor.matmul(rrep_ps, onescol, rrow,
                                         start=True, stop=True)
                        nc.vector.tensor_mul(
                            onTp[hh // 2][(hh % 2) * C:(hh % 2) * C + C, tsl],
                            osb[0:C, :], rrep_ps)
                        if dbg and t2 == 0 and hh == 0:
                            nc.sync.dma_start(out=dbg_d["d_osb"][:], in_=osb)
                            rrep_sb = nmp.tile([C, D], F32, tag="rrep_dbg",
                                               name="rrep_dbg", bufs=1)
                            nc.vector.tensor_copy(rrep_sb, rrep_ps)
                            nc.sync.dma_start(out=dbg_d["d_rrep"][:], in_=rrep_sb)

            if dbg:
                nc.sync.dma_start(out=dbg_d["d_onT0"][:], in_=onTp[0][0:C, :])

            # ---- phase 3: output projection ----
            for tqc in range(8):
                ps = psp.tile([P, D], F32, tag="ob", name="out_ps")
                for j in range(NH // 2):
                    nc.tensor.matmul(
                        ps, onTp[j][:, tqc * P:(tqc + 1) * P], wo[j],
                        start=(j == 0),
                        stop=(j == NH // 2 - 1 and not use_bias))
                if use_bias:
                    nc.tensor.matmul(
                        ps, ones[:, 0:P], bias_sb[0:1, 3 * D:4 * D],
                        start=False, stop=True)
                osb2 = otp.tile([P, D], F32, tag="outsb", name="outsb")
                nc.vector.tensor_copy(osb2, ps)
                nc.sync.dma_start(out=out_d[tqc * P:(tqc + 1) * P, :], in_=osb2)

    nc.compile()
    return nc


def _prep(q, k, v, mask, Wq, bq, Wk, bk, Wv, bv, Wo, bo):
    q = np.asarray(q, np.float32)
    k = np.asarray(k, np.float32)
    v = np.asarray(v, np.float32)
    mask = np.asarray(mask)
    wqt = np.ascontiguousarray(np.asarray(Wq, np.float32).T.astype(np.float16))
    wkt = np.ascontiguousarray(np.asarray(Wk, np.float32).T.astype(np.float16))
    wvt = np.ascontiguousarray(np.asarray(Wv, np.float32).T.astype(np.float16))
    wot = np.ascontiguousarray(np.asarray(Wo, np.float32).T.astype(np.float16))
    biascol = np.concatenate([
        np.asarray(bq, np.float32).reshape(4, P).T,
        np.asarray(bk, np.float32).reshape(4, P).T], axis=1)
    biascol = np.ascontiguousarray(biascol, dtype=np.float32)
    biases = np.concatenate(
        [np.asarray(x, np.float32) for x in (bq, bk, bv, bo)]
        + [np.ones(TQ, np.float32)]).reshape(1, 4 * D + TQ).astype(np.float16)

    sels = [np.flatnonzero(mask[b]) for b in range(B)]
    kmax = max(1, max(len(s) for s in sels))
    KP = ((kmax + P - 1) // P) * P

    in_maps = []
    for core in range(N_CORES):
        b, half = divmod(core, 2)
        sel = sels[b]
        ns = len(sel)
        kt = np.zeros((D, KP), np.float16)
        kt[:, :ns] = k[b, sel, :].T
        vt = np.zeros((D, KP), np.float16)
        vt[:, :ns] = v[b, sel, :].T
        valid = np.zeros((KP, NH, 1), np.float16)
        valid[:ns] = 1.0
        validc = np.zeros((KP, 1), np.float32)
        validc[:ns] = 1.0
        qt = np.ascontiguousarray(
            q[b, half * TQ:(half + 1) * TQ, :].T.astype(np.float16))
        in_maps.append(dict(
            qt=qt, kt=kt, vt=vt, wqt=wqt, wkt=wkt, wvt=wvt, wot=wot,
            biases=biases, valid=valid, validc=validc, biascol=biascol))
    return KP, in_maps


def kernel(q, k, v, mask, Wq, bq, Wk, bk, Wv, bv, Wo, bo, _bench=[None]):
    KP, in_maps = _prep(q, k, v, mask, Wq, bq, Wk, bk, Wv, bv, Wo, bo)
    use_bias = any(
        bool(np.any(np.asarray(x))) for x in (bq, bk, bv, bo))
    nc = _build(KP, False, use_bias)
    res = run_bass_kernel_spmd(nc, in_maps, list(range(N_CORES)))
    _bench[0] = res
    out = np.empty((B, T, D), np.float32)
    for core in range(N_CORES):
        b, half = divmod(core, 2)
        out[b, half * TQ:(half + 1) * TQ, :] = res.results[core]["out"]
    return out
